# revision 41
# baseline (speedup 1.0000x reference)
"""Divergence-free RBF kernel Gram matrix on 8 Trainium2 NeuronCores.

Math: for d=2, with scaled coords x' = x*exp(-ll/2):
  dx = x0_i - y0_j, dy = x1_i - y1_j, r2 = dx^2 + dy^2, e = exp(-r2/2)
  K[2i+0, 2j+0] = e * (1 - dy^2)
  K[2i+0, 2j+1] = K[2i+1, 2j+0] = e * dx*dy
  K[2i+1, 2j+1] = e * (1 - dx^2)

The off-diagonal channel is EXACTLY duplicated in the output, so the device
computes and stores only the 3 unique planes (c00, dxdy, c11), each (n, m),
as bf16; the host interleaves them into the (2n, 2m) f32 Gram matrix.
This cuts HBM write traffic 8x vs the dense f32 output (4 channels f32 ->
3 planes bf16) while norm rel err stays ~1e-3 (gate is 2e-2).

Each polynomial factor is low-rank in the basis {1, x0, x1, x0*x1, x0^2,
x1^2} (K=6): host precomputes the X-side basis L [6, n] and per-plane
Y-side coefficients R_p [6, m]; fp32-grade matmul precision comes from a
hi/lo bf16 split stacked to K=18: [Lhi;Llo;Lhi].T @ [Rhi;Rhi;Rlo].

Engine split (per 128-row x 512-col unit; all four engines land ~1.2us):
  PE  : 4 concurrent K=18 matmul streams via 32-row array tiling
        (tile_position groups 0/32/64/96 = c00/dxdy/c11/r2); W and R are
        packed at partition offsets 0/32/64/96 of single SBUF tiles.
  ACT : e = exp(-r2/2) (bf16) + evict dxdy plane PSUM->SBUF bf16
  DVE : one fused multiply (c00|c11 in one [128,1024] PSUM tile) x e
  GPS : dxdy (SBUF bf16) x e -> out
  DMA : merged [c00|dxdy|c11] segment stores; i-block 0 drains in 1024-col
        quarters (early stream start), later i-blocks in 2048-col halves.
PSUM is exactly 8 banks: (mcc 2 + md 1 + r2 1) banks x ring-2 -- the ring-2
double buffering lets PE run a unit ahead of the multiplies.

Sharding: rows of X (n axis) split across 8 cores, 512 each -> each core
writes 3 planes of 512 rows of the (4096, 4096)-per-plane output. No
communication.
"""

import numpy as np
import ml_dtypes

N = 4096          # X rows
M = 4096          # Y rows
D = 2
NCORES = 8
NPC = N // NCORES  # 512 X rows per core
IB = 128           # i-block = partition count
NIB = NPC // IB    # 4 i-blocks per core
JQ = 512           # j-chunk per unit (1 PSUM bank per plane matmul)
NJQ = M // JQ      # 8 j-chunks
KST = 18           # stacked contraction dim (3 x 6 basis rows)
USE_GPS = True     # dxdy multiply on GpSimd (else DVE does all 3 planes)

_cache = {}


def _hi_lo(a):
    bf = ml_dtypes.bfloat16
    hi = a.astype(bf)
    lo = (a - hi.astype(np.float64)).astype(bf)
    return hi, lo


def _prepare_inputs(X, Y, log_length_scale):
    s = float(np.exp(-0.5 * np.float64(np.asarray(log_length_scale).reshape(-1)[0])))
    xs = np.asarray(X, dtype=np.float64).reshape(N, D) * s
    ys = np.asarray(Y, dtype=np.float64).reshape(M, D) * s
    x0, x1 = xs[:, 0], xs[:, 1]
    y0, y1 = ys[:, 0], ys[:, 1]
    one_n, zero_m, one_m = np.ones(N), np.zeros(M), np.ones(M)

    # X-side basis [6, N]: rows {1, x0, x1, x0*x1, x0^2, x1^2}
    L = np.stack([one_n, x0, x1, x0 * x1, x0 ** 2, x1 ** 2])

    # Y-side coefficient columns [6, M] per output plane
    c00 = np.stack([1 - y1 ** 2, zero_m, 2 * y1, zero_m, zero_m, -one_m])
    cdd = np.stack([y0 * y1, -y1, -y0, one_m, zero_m, zero_m])
    c11 = np.stack([1 - y0 ** 2, 2 * y0, zero_m, zero_m, -one_m, zero_m])
    cr2 = np.stack([y0 ** 2 + y1 ** 2, -2 * y0, -2 * y1, zero_m, one_m, one_m])

    Lh, Ll = _hi_lo(L)
    Lst = np.concatenate([Lh, Ll, Lh], axis=0)  # (18, N)

    bf = ml_dtypes.bfloat16
    W = np.zeros((128, N), dtype=bf)
    R = np.zeros((128, M), dtype=bf)
    for a, plane in enumerate([c00, cdd, c11, cr2]):
        Rh, Rl = _hi_lo(plane)
        W[32 * a:32 * a + KST, :] = Lst
        R[32 * a:32 * a + KST, :] = np.concatenate([Rh, Rh, Rl], axis=0)
    return np.ascontiguousarray(W), np.ascontiguousarray(R)


def _build_module(bass_cls=None, **bass_kw):
    from concourse import bacc, mybir
    import concourse.tile as tile

    bf16 = mybir.dt.bfloat16
    f32 = mybir.dt.float32
    Exp = mybir.ActivationFunctionType.Exp

    if bass_cls is None:
        bass_cls = bacc.Bacc
    nc = bass_cls("TRN2", target_bir_lowering=False, debug=False,
                  enable_asserts=False, **bass_kw)
    # single packed input [w-cols | r-cols] so the first chunk carries the
    # weights AND the first 512 r columns in one DMA (shortest head)
    in_d = nc.dram_tensor("inA", [128, NPC + M], bf16, kind="ExternalInput")
    # rows [0:512) plane c00, [512:1024) dxdy, [1024:1536) c11
    out_d = nc.dram_tensor("out", [3 * NPC, M], bf16, kind="ExternalOutput")

    with tile.TileContext(nc) as tc:
        with (
            tc.tile_pool(name="const", bufs=1) as cpool,
            tc.tile_pool(name="outp", bufs=4) as opool,
            tc.tile_pool(name="ep", bufs=6) as epool,
            tc.tile_pool(name="dp", bufs=6) as dpool,
            tc.tile_pool(name="ps_r2", bufs=2, space="PSUM") as rpool,
            tc.tile_pool(name="ps_mm", bufs=2, space="PSUM") as mpool,
        ):
            # chunk 0 = [w | r 0:512]; later chunks grow as head pressure
            # relaxes. All boundaries 512-aligned in r-space.
            bounds = [0, NPC + 512, NPC + 1536, NPC + 2560, NPC + 4096]
            chunks = []
            for ci in range(len(bounds) - 1):
                b0, b1 = bounds[ci], bounds[ci + 1]
                t = cpool.tile([128, b1 - b0], bf16, tag=f"in{ci}")
                nc.sync.dma_start(out=t[:], in_=in_d[:, b0:b1])
                chunks.append((b0, b1, t))

            def in_slice(grp, g0, width):
                for b0, b1, t in chunks:
                    if b0 <= g0 < b1:
                        assert g0 + width <= b1, (g0, width, b0, b1)
                        return t[32 * grp:32 * grp + KST,
                                 g0 - b0:g0 - b0 + width]
                raise AssertionError(g0)

            # [512, 3, 4096] view: iteration (row-in-plane, plane, col)
            # matches the [part, t, m] order of the c00|c11 SBUF tile.
            out_v = out_d.ap().rearrange("(t r) m -> r t m", t=3)

            # i-block 0 (the one that drains in small quarter-stores) goes
            # LAST: the kernel's tail is then a 0.75MB store instead of
            # 1.5MB, ~2.3us shorter. Mid-kernel DMA slack absorbs the
            # slightly later drain start.
            units = [(ib, q) for ib in (1, 2, 3, 0) for q in range(NJQ)]
            e_of = {}

            def mm(out_ap, grp, ib, q, s2):
                c0 = q * JQ + s2 * 512
                nc.tensor.matmul(
                    out_ap,
                    in_slice(grp, ib * IB, IB),
                    in_slice(grp, NPC + c0, 512),
                    start=True, stop=True, tile_position=(32 * grp, 0))

            def issue_r2exp(idx):
                uib, uq = units[idx]
                r2q = rpool.tile([IB, JQ], f32, tag="r2")
                for s2 in range(JQ // 512):
                    mm(r2q[:, s2 * 512:(s2 + 1) * 512], 3, uib, uq, s2)
                e = epool.tile([IB, JQ], bf16, tag="e")
                nc.scalar.activation(e[:], r2q[:], Exp, scale=-0.5)
                e_of[idx] = e

            issue_r2exp(0)
            # flush segments: i-block 0 drains in 1024-col quarters so the
            # output DMA stream starts ~4us earlier; later i-blocks use
            # 2048-col halves (bigger, more efficient stores).
            seg_of = {}
            for ib_ in range(NIB):
                segs = ([(0, 1), (2, 3), (4, 5), (6, 7)] if ib_ == 0
                        else [(0, 3), (4, 7)])
                for q0, q1 in segs:
                    for q_ in range(q0, q1 + 1):
                        seg_of[(ib_, q_)] = (q0, q1)
            ov = None
            for idx, (ib, q) in enumerate(units):
                e = e_of.pop(idx)
                q0, q1 = seg_of[(ib, q)]
                SC = (q1 - q0 + 1) * JQ
                if q == q0:
                    # merged [c00 | dxdy | c11] tile -> one DMA per segment
                    ot = opool.tile([IB, 3 * SC], bf16,
                                    tag=f"o{'S' if SC == 1024 else 'L'}")
                    ov = ot[:].rearrange("p (t m) -> p t m", t=3)
                qc = (q - q0) * JQ
                # c00 into cols [0:JQ), c11 into [JQ:2JQ) of one PSUM tile
                mcc = mpool.tile([IB, 2 * JQ], f32, tag="mcc")
                for ai, grp in enumerate((0, 2)):
                    for s2 in range(JQ // 512):
                        mm(mcc[:, ai * JQ + s2 * 512:ai * JQ + (s2 + 1) * 512],
                           grp, ib, q, s2)
                md = mpool.tile([IB, JQ], f32, tag="md")
                for s2 in range(JQ // 512):
                    mm(md[:, s2 * 512:(s2 + 1) * 512], 1, ib, q, s2)
                if USE_GPS:
                    dsb = dpool.tile([IB, JQ], bf16, tag="d")
                    nc.scalar.copy(dsb[:], md[:])
                # next unit's r2 matmuls + exp pipeline behind this unit's
                # evict (ACT) and plane matmuls (PE group 96 is free)
                if idx + 1 < len(units):
                    issue_r2exp(idx + 1)
                if USE_GPS:
                    nc.gpsimd.tensor_mul(ov[:, 1, qc:qc + JQ], dsb[:], e[:])
                else:
                    nc.vector.tensor_mul(ov[:, 1, qc:qc + JQ], md[:], e[:])
                # DVE: fused (c00|c11) x e -> bf16 out tile slices 0 and 2
                eb = e[:].unsqueeze(1).broadcast_to([IB, 2, JQ])
                nc.vector.tensor_mul(
                    ov[:, 0:3:2, qc:qc + JQ],
                    mcc[:].rearrange("p (t j) -> p t j", t=2),
                    eb)
                if q == q1:
                    i0 = ib * IB
                    c0 = q0 * JQ
                    nc.sync.dma_start(
                        out=out_v[i0:i0 + IB, :, c0:c0 + SC],
                        in_=ov)
    nc.finalize()
    return nc


def _run(X, Y, log_length_scale, trace=False):
    from concourse.bass_utils import run_bass_kernel_spmd

    W, R = _prepare_inputs(X, Y, log_length_scale)
    if "nc" not in _cache:
        _cache["nc"] = _build_module()
    nc = _cache["nc"]
    in_maps = [
        {
            "inA": np.ascontiguousarray(np.concatenate(
                [W[:, c * NPC:(c + 1) * NPC], R], axis=1)),
        }
        for c in range(NCORES)
    ]
    res = run_bass_kernel_spmd(nc, in_maps, core_ids=list(range(NCORES)),
                               trace=trace)
    big = np.empty((N, 2, M, 2), dtype=np.float32)
    for c in range(NCORES):
        pc = np.asarray(res.results[c]["out"]).reshape(3, NPC, M)
        sl = slice(c * NPC, (c + 1) * NPC)
        c00 = pc[0].astype(np.float32)
        cdd = pc[1].astype(np.float32)
        c11 = pc[2].astype(np.float32)
        big[sl, 0, :, 0] = c00
        big[sl, 0, :, 1] = cdd
        big[sl, 1, :, 0] = cdd
        big[sl, 1, :, 1] = c11
    return big.reshape(1, 2 * N, 2 * M), res


def kernel(X, Y, log_length_scale):
    out, _ = _run(np.asarray(X), np.asarray(Y), np.asarray(log_length_scale))
    return out


# revision 42
# speedup vs baseline: 1.1287x; 1.1287x over previous
"""Divergence-free RBF kernel Gram matrix on 8 Trainium2 NeuronCores.

Math: for d=2, with scaled coords x' = x*exp(-ll/2):
  dx = x0_i - y0_j, dy = x1_i - y1_j, r2 = dx^2 + dy^2, e = exp(-r2/2)
  K[2i+0, 2j+0] = e * (1 - dy^2)
  K[2i+0, 2j+1] = K[2i+1, 2j+0] = e * dx*dy
  K[2i+1, 2j+1] = e * (1 - dx^2)

The off-diagonal channel is EXACTLY duplicated in the output, so the device
computes and stores only the 3 unique planes (c00, dxdy, c11), each (n, m),
as bf16; the host interleaves them into the (2n, 2m) f32 Gram matrix.
This cuts HBM write traffic 8x vs the dense f32 output (4 channels f32 ->
3 planes bf16) while norm rel err stays ~1e-3 (gate is 2e-2).

Each polynomial factor is low-rank in the basis {1, x0, x1, x0*x1, x0^2,
x1^2} (K=6): host precomputes the X-side basis L [6, n] and per-plane
Y-side coefficients R_p [6, m]; fp32-grade matmul precision comes from a
hi/lo bf16 split stacked to K=18: [Lhi;Llo;Lhi].T @ [Rhi;Rhi;Rlo].

Engine split (per 128-row x 1024-col unit):
  PE  : 4 concurrent K=18 matmul streams via 32-row array tiling
        (tile_position groups 0/32/64/96 = c00/dxdy/c11/r2); W and R are
        packed at partition offsets 0/32/64/96 of single SBUF tiles.
  ACT : e = exp(-r2/2) (bf16) + evict dxdy plane PSUM->SBUF bf16
  DVE : one fused multiply (c00|c11 in one [128,2048] PSUM tile) x e
  GPS : dxdy (SBUF bf16) x e -> out
  DMA : per i-block, 2MB (c00+c11) + 1MB (dxdy) bf16 stores

Sharding: rows of X (n axis) split across 8 cores, 512 each -> each core
writes 3 planes of 512 rows of the (4096, 4096)-per-plane output. No
communication.
"""

import numpy as np
import ml_dtypes

N = 4096          # X rows
M = 4096          # Y rows
D = 2
NCORES = 8
NPC = N // NCORES  # 512 X rows per core
IB = 128           # i-block = partition count
NIB = NPC // IB    # 4 i-blocks per core
JQ = 512           # j-chunk per unit (1 PSUM bank per plane matmul)
NJQ = M // JQ      # 8 j-chunks
KST = 18           # stacked contraction dim (3 x 6 basis rows)
USE_GPS = True     # dxdy multiply on GpSimd (else DVE does all 3 planes)

_cache = {}


def _hi_lo(a):
    bf = ml_dtypes.bfloat16
    hi = a.astype(bf)
    lo = (a - hi.astype(np.float64)).astype(bf)
    return hi, lo


def _prepare_inputs(X, Y, log_length_scale):
    s = float(np.exp(-0.5 * np.float64(np.asarray(log_length_scale).reshape(-1)[0])))
    xs = np.asarray(X, dtype=np.float64).reshape(N, D) * s
    ys = np.asarray(Y, dtype=np.float64).reshape(M, D) * s
    x0, x1 = xs[:, 0], xs[:, 1]
    y0, y1 = ys[:, 0], ys[:, 1]
    one_n, zero_m, one_m = np.ones(N), np.zeros(M), np.ones(M)

    # X-side basis [6, N]: rows {1, x0, x1, x0*x1, x0^2, x1^2}
    L = np.stack([one_n, x0, x1, x0 * x1, x0 ** 2, x1 ** 2])

    # Y-side coefficient columns [6, M] per output plane
    c00 = np.stack([1 - y1 ** 2, zero_m, 2 * y1, zero_m, zero_m, -one_m])
    cdd = np.stack([y0 * y1, -y1, -y0, one_m, zero_m, zero_m])
    c11 = np.stack([1 - y0 ** 2, 2 * y0, zero_m, zero_m, -one_m, zero_m])
    cr2 = np.stack([y0 ** 2 + y1 ** 2, -2 * y0, -2 * y1, zero_m, one_m, one_m])

    Lh, Ll = _hi_lo(L)
    Lst = np.concatenate([Lh, Ll, Lh], axis=0)  # (18, N)

    bf = ml_dtypes.bfloat16
    W = np.zeros((128, N), dtype=bf)
    R = np.zeros((128, M), dtype=bf)
    for a, plane in enumerate([c00, cdd, c11, cr2]):
        Rh, Rl = _hi_lo(plane)
        W[32 * a:32 * a + KST, :] = Lst
        R[32 * a:32 * a + KST, :] = np.concatenate([Rh, Rh, Rl], axis=0)
    return np.ascontiguousarray(W), np.ascontiguousarray(R)


def _build_module(bass_cls=None, **bass_kw):
    from concourse import bacc, mybir
    import concourse.tile as tile

    bf16 = mybir.dt.bfloat16
    f32 = mybir.dt.float32
    Exp = mybir.ActivationFunctionType.Exp

    if bass_cls is None:
        bass_cls = bacc.Bacc
    nc = bass_cls("TRN2", target_bir_lowering=False, debug=False,
                  enable_asserts=False, **bass_kw)
    # single packed input [w-cols | r-cols] so the first chunk carries the
    # weights AND the first 512 r columns in one DMA (shortest head)
    in_d = nc.dram_tensor("inA", [128, NPC + M], bf16, kind="ExternalInput")
    # rows [0:512) plane c00, [512:1024) dxdy, [1024:1536) c11
    out_d = nc.dram_tensor("out", [3 * NPC, M], bf16, kind="ExternalOutput")

    with tile.TileContext(nc) as tc:
        with (
            tc.tile_pool(name="const", bufs=1) as cpool,
            tc.tile_pool(name="outp", bufs=4) as opool,
            tc.tile_pool(name="ep", bufs=6) as epool,
            tc.tile_pool(name="dp", bufs=6) as dpool,
            tc.tile_pool(name="ps_r2", bufs=2, space="PSUM") as rpool,
            tc.tile_pool(name="ps_mm", bufs=2, space="PSUM") as mpool,
        ):
            # chunk 0 = [w | r 0:512]; later chunks grow as head pressure
            # relaxes. All boundaries 512-aligned in r-space.
            bounds = [0, NPC + 512, NPC + 1536, NPC + 2560, NPC + 4096]
            chunks = []
            for ci in range(len(bounds) - 1):
                b0, b1 = bounds[ci], bounds[ci + 1]
                t = cpool.tile([128, b1 - b0], bf16, tag=f"in{ci}")
                nc.sync.dma_start(out=t[:], in_=in_d[:, b0:b1])
                chunks.append((b0, b1, t))

            def in_slice(grp, g0, width):
                for b0, b1, t in chunks:
                    if b0 <= g0 < b1:
                        assert g0 + width <= b1, (g0, width, b0, b1)
                        return t[32 * grp:32 * grp + KST,
                                 g0 - b0:g0 - b0 + width]
                raise AssertionError(g0)

            # [512, 3, 4096] view: iteration (row-in-plane, plane, col)
            # matches the [part, t, m] order of the c00|c11 SBUF tile.
            out_v = out_d.ap().rearrange("(t r) m -> r t m", t=3)

            units = [(ib, q) for ib in range(NIB) for q in range(NJQ)]
            e_of = {}

            def mm(out_ap, grp, ib, q, s2):
                c0 = q * JQ + s2 * 512
                nc.tensor.matmul(
                    out_ap,
                    in_slice(grp, ib * IB, IB),
                    in_slice(grp, NPC + c0, 512),
                    start=True, stop=True, tile_position=(32 * grp, 0))

            def issue_r2exp(idx):
                uib, uq = units[idx]
                r2q = rpool.tile([IB, JQ], f32, tag="r2")
                for s2 in range(JQ // 512):
                    mm(r2q[:, s2 * 512:(s2 + 1) * 512], 3, uib, uq, s2)
                e = epool.tile([IB, JQ], bf16, tag="e")
                nc.scalar.activation(e[:], r2q[:], Exp, scale=-0.5)
                e_of[idx] = e

            issue_r2exp(0)
            # flush segments: i-block 0 drains in 1024-col quarters so the
            # output DMA stream starts ~4us earlier; later i-blocks use
            # 2048-col halves (bigger, more efficient stores).
            seg_of = {}
            for ib_ in range(NIB):
                segs = ([(0, 0), (1, 1), (2, 3), (4, 5), (6, 7)]
                        if ib_ == 0 else [(0, 3), (4, 7)])
                for q0, q1 in segs:
                    for q_ in range(q0, q1 + 1):
                        seg_of[(ib_, q_)] = (q0, q1)
            ov = None
            for idx, (ib, q) in enumerate(units):
                e = e_of.pop(idx)
                q0, q1 = seg_of[(ib, q)]
                SC = (q1 - q0 + 1) * JQ
                if q == q0:
                    # merged [c00 | dxdy | c11] tile -> one DMA per segment
                    ot = opool.tile([IB, 3 * SC], bf16, tag=f"o{SC}")
                    ov = ot[:].rearrange("p (t m) -> p t m", t=3)
                qc = (q - q0) * JQ
                # c00 into cols [0:JQ), c11 into [JQ:2JQ) of one PSUM tile
                mcc = mpool.tile([IB, 2 * JQ], f32, tag="mcc")
                for ai, grp in enumerate((0, 2)):
                    for s2 in range(JQ // 512):
                        mm(mcc[:, ai * JQ + s2 * 512:ai * JQ + (s2 + 1) * 512],
                           grp, ib, q, s2)
                md = mpool.tile([IB, JQ], f32, tag="md")
                for s2 in range(JQ // 512):
                    mm(md[:, s2 * 512:(s2 + 1) * 512], 1, ib, q, s2)
                if USE_GPS:
                    dsb = dpool.tile([IB, JQ], bf16, tag="d")
                    nc.scalar.copy(dsb[:], md[:])
                # next unit's r2 matmuls + exp pipeline behind this unit's
                # evict (ACT) and plane matmuls (PE group 96 is free)
                if idx + 1 < len(units):
                    issue_r2exp(idx + 1)
                if USE_GPS:
                    nc.gpsimd.tensor_mul(ov[:, 1, qc:qc + JQ], dsb[:], e[:])
                else:
                    nc.vector.tensor_mul(ov[:, 1, qc:qc + JQ], md[:], e[:])
                # DVE: fused (c00|c11) x e -> bf16 out tile slices 0 and 2
                eb = e[:].unsqueeze(1).broadcast_to([IB, 2, JQ])
                nc.vector.tensor_mul(
                    ov[:, 0:3:2, qc:qc + JQ],
                    mcc[:].rearrange("p (t j) -> p t j", t=2),
                    eb)
                if q == q1:
                    i0 = ib * IB
                    c0 = q0 * JQ
                    nc.sync.dma_start(
                        out=out_v[i0:i0 + IB, :, c0:c0 + SC],
                        in_=ov)
    nc.finalize()
    return nc


def _run(X, Y, log_length_scale, trace=False):
    from concourse.bass_utils import run_bass_kernel_spmd

    W, R = _prepare_inputs(X, Y, log_length_scale)
    if "nc" not in _cache:
        _cache["nc"] = _build_module()
    nc = _cache["nc"]
    in_maps = [
        {
            "inA": np.ascontiguousarray(np.concatenate(
                [W[:, c * NPC:(c + 1) * NPC], R], axis=1)),
        }
        for c in range(NCORES)
    ]
    res = run_bass_kernel_spmd(nc, in_maps, core_ids=list(range(NCORES)),
                               trace=trace)
    big = np.empty((N, 2, M, 2), dtype=np.float32)
    for c in range(NCORES):
        pc = np.asarray(res.results[c]["out"]).reshape(3, NPC, M)
        sl = slice(c * NPC, (c + 1) * NPC)
        c00 = pc[0].astype(np.float32)
        cdd = pc[1].astype(np.float32)
        c11 = pc[2].astype(np.float32)
        big[sl, 0, :, 0] = c00
        big[sl, 0, :, 1] = cdd
        big[sl, 1, :, 0] = cdd
        big[sl, 1, :, 1] = c11
    return big.reshape(1, 2 * N, 2 * M), res


def kernel(X, Y, log_length_scale):
    out, _ = _run(np.asarray(X), np.asarray(Y), np.asarray(log_length_scale))
    return out
